# revision 2
# baseline (speedup 1.0000x reference)
"""Trainium2 Bass kernel for the AgreemFlat top-k masking model.

Reference computation (per batch row b):
    sims[b, n]  = <sim_stance_emb[b], sim_body_emb[b, n]>         n in [0, 64)
    top5        = top_k(sims[b], 5).indices                        (descending)
    xx[b]       = [nli_stance_emb[b] | nli_body_emb[b, top5].ravel()]   (4608)
    out[b]      = relu(xx @ W1.T + b1) @ W2.T + b2                 (4 classes)

Sharding: pure data-parallel over B=4096 -> 8 cores x 512 rows.
Weights replicated; W1^T / W2^T / b1 are pre-arranged host-side into
partition-major layouts so the device only does straight DMA loads.

Per-core dataflow (chunks of 128 batch rows on partitions):
  DMA sim_body stream -> DVE tensor_tensor_reduce (fp32 dot products)
  -> vector.max / max_index (top-8, take 5) -> indirect DMA gather
  -> PE transpose (+ ACT cast to bf16) -> PE matmul FC1 (bf16, fp32 psum)
  -> ACT fused bias+relu -> PE matmul FC2 -> bias add -> PE transpose -> DMA out.
"""

import numpy as np
import ml_dtypes

import concourse.bass as bass
import concourse.mybir as mybir
import concourse.tile as tile
from concourse import bacc
from concourse.masks import make_identity

P = 128
N = 64            # body sentences per row
D = 768           # embedding dim (sim and nli)
KK = 5            # top-k
H1 = 1024         # fc1 output dim
NC_OUT = 4        # classes
FAN1 = (KK + 1) * D          # 4608 = fc1 fan-in
KCH = FAN1 // P              # 36 contraction chunks
HCH = H1 // P                # 8 fc1-output chunks
N_CORES = 8
B_FULL = 4096
BL = B_FULL // N_CORES       # 512 rows per core
NSUB = 8                     # body n-sentences per streamed subtile


def build(bl=BL):
    chunks = bl // P
    nsubs = N // NSUB
    fp32 = mybir.dt.float32
    bf16 = mybir.dt.bfloat16
    u32 = mybir.dt.uint32

    nc = bacc.Bacc("TRN2", target_bir_lowering=False)

    sim_stance = nc.dram_tensor("sim_stance", [bl, D], fp32, kind="ExternalInput")
    nli_stance = nc.dram_tensor("nli_stance", [bl, D], fp32, kind="ExternalInput")
    sim_body = nc.dram_tensor("sim_body", [bl, N * D], fp32, kind="ExternalInput")
    nli_body = nc.dram_tensor("nli_body", [bl * N, D], fp32, kind="ExternalInput")
    w1t = nc.dram_tensor("w1t", [P, KCH * H1], bf16, kind="ExternalInput")
    w2t = nc.dram_tensor("w2t", [P, HCH * NC_OUT], bf16, kind="ExternalInput")
    b1t = nc.dram_tensor("b1t", [P, HCH], fp32, kind="ExternalInput")
    b2c = nc.dram_tensor("b2c", [NC_OUT, 1], fp32, kind="ExternalInput")
    out = nc.dram_tensor("out", [bl, NC_OUT], fp32, kind="ExternalOutput")

    with tile.TileContext(nc) as tc:
        with (
            tc.tile_pool(name="wpool", bufs=1) as wpool,
            tc.tile_pool(name="bodyp", bufs=2) as bodyp,
            tc.tile_pool(name="stancep", bufs=2) as stancep,
            tc.tile_pool(name="simsp", bufs=2) as simsp,
            tc.tile_pool(name="topkp", bufs=2) as topkp,
            tc.tile_pool(name="gp", bufs=2) as gp,
            tc.tile_pool(name="xxtp", bufs=2) as xxtp,
            tc.tile_pool(name="yp", bufs=2) as yp,
            tc.tile_pool(name="op", bufs=2) as op,
            tc.tile_pool(name="pt", bufs=2, space="PSUM") as pt_pool,
            tc.tile_pool(name="py", bufs=2, space="PSUM") as py_pool,
            tc.tile_pool(name="po", bufs=2, space="PSUM") as po_pool,
        ):
            # ---- setup: weights, biases, identity ----
            w1t_sb = wpool.tile([P, KCH * H1], bf16)
            nc.sync.dma_start(w1t_sb[:], w1t[:, :])
            w2t_sb = wpool.tile([P, HCH * NC_OUT], bf16)
            nc.sync.dma_start(w2t_sb[:], w2t[:, :])
            b1t_sb = wpool.tile([P, HCH], fp32)
            nc.sync.dma_start(b1t_sb[:], b1t[:, :])
            b2_sb = wpool.tile([NC_OUT, 1], fp32)
            nc.sync.dma_start(b2_sb[:], b2c[:, :])
            ident = wpool.tile([P, P], fp32)
            make_identity(nc, ident[:])

            for ch in range(chunks):
                r0 = ch * P
                # ---- stance tiles for this chunk ----
                sstance = stancep.tile([P, D], fp32, tag="sstance")
                nc.sync.dma_start(sstance[:], sim_stance[r0:r0 + P, :])
                nstance = stancep.tile([P, D], fp32, tag="nstance")
                nc.sync.dma_start(nstance[:], nli_stance[r0:r0 + P, :])

                # ---- sims: stream body, fused multiply+reduce on DVE ----
                sims = simsp.tile([P, N], fp32)
                for s in range(nsubs):
                    body = bodyp.tile([P, NSUB * D], fp32, tag="body")
                    nc.sync.dma_start(
                        body[:],
                        sim_body[r0:r0 + P, s * NSUB * D:(s + 1) * NSUB * D],
                    )
                    for j in range(NSUB):
                        nrow = s * NSUB + j
                        seg = body[:, j * D:(j + 1) * D]
                        nc.vector.scalar_tensor_tensor(
                            out=seg,
                            in0=seg,
                            scalar=1.0,
                            in1=sstance[:],
                            op0=mybir.AluOpType.mult,
                            op1=mybir.AluOpType.mult,
                            accum_out=sims[:, nrow:nrow + 1],
                        )

                # ---- top-5 of 64 (descending, matches jax.lax.top_k) ----
                max8 = topkp.tile([P, 8], fp32, tag="max8")
                nc.vector.max(out=max8[:], in_=sims[:])
                idx8 = topkp.tile([P, 8], u32, tag="idx8")
                nc.vector.max_index(out=idx8[:], in_max=max8[:], in_values=sims[:])

                # global row index into nli_body rows: (r0 + p) * 64 + idx
                rowbase = topkp.tile([P, 8], u32, tag="rowbase")
                nc.gpsimd.iota(
                    rowbase[:], [[0, 8]], base=r0 * N, channel_multiplier=N,
                )
                rows = topkp.tile([P, 8], u32, tag="rows")
                nc.vector.tensor_tensor(
                    out=rows[:], in0=idx8[:], in1=rowbase[:],
                    op=mybir.AluOpType.add,
                )

                # ---- gather top-5 nli body embeddings: [128, 5*768] ----
                xg = gp.tile([P, KK * D], fp32)
                for k in range(KK):
                    nc.gpsimd.indirect_dma_start(
                        out=xg[:, k * D:(k + 1) * D],
                        out_offset=None,
                        in_=nli_body[:, :],
                        in_offset=bass.IndirectOffsetOnAxis(
                            ap=rows[:, k:k + 1], axis=0
                        ),
                    )

                # ---- xx^T in bf16 via PE transpose + ACT cast ----
                # xx = [nli_stance | gathered] : [128, 4608]
                xxt = xxtp.tile([P, FAN1], bf16)
                for t in range(KCH):
                    if t < D // P:
                        src = nstance[:, t * P:(t + 1) * P]
                    else:
                        g = t - D // P
                        src = xg[:, g * P:(g + 1) * P]
                    ptile = pt_pool.tile([P, P], fp32, tag="pt")
                    nc.tensor.transpose(out=ptile[:], in_=src, identity=ident[:])
                    nc.scalar.activation(
                        out=xxt[:, t * P:(t + 1) * P], in_=ptile[:],
                        func=mybir.ActivationFunctionType.Copy,
                    )

                # ---- FC1 (bf16): y^T[h, b] = W1 @ xx^T, fused bias+relu ----
                yt = yp.tile([P, H1], bf16)
                for m in range(HCH):
                    py = py_pool.tile([P, P], fp32, tag="py")
                    for c in range(KCH):
                        nc.tensor.matmul(
                            out=py[:],
                            lhsT=w1t_sb[:, c * H1 + m * P:c * H1 + (m + 1) * P],
                            rhs=xxt[:, c * P:(c + 1) * P],
                            start=(c == 0),
                            stop=(c == KCH - 1),
                        )
                    nc.scalar.activation(
                        out=yt[:, m * P:(m + 1) * P], in_=py[:],
                        func=mybir.ActivationFunctionType.Relu,
                        bias=b1t_sb[:, m:m + 1],
                    )

                # ---- FC2: o^T[4, b] = W2 @ y^T ----
                po = po_pool.tile([NC_OUT, P], fp32, tag="po")
                for m in range(HCH):
                    nc.tensor.matmul(
                        out=po[:],
                        lhsT=w2t_sb[:, m * NC_OUT:(m + 1) * NC_OUT],
                        rhs=yt[:, m * P:(m + 1) * P],
                        start=(m == 0),
                        stop=(m == HCH - 1),
                    )
                osb = op.tile([NC_OUT, P], fp32, tag="osb")
                nc.vector.tensor_tensor(
                    out=osb[:], in0=po[:],
                    in1=b2_sb[:].to_broadcast([NC_OUT, P]),
                    op=mybir.AluOpType.add,
                )

                # ---- transpose [4, 128] -> [128, 4] and store ----
                pout = pt_pool.tile([P, NC_OUT], fp32, tag="pout")
                nc.tensor.transpose(
                    out=pout[:], in_=osb[:], identity=ident[:NC_OUT, :NC_OUT]
                )
                out_sb = op.tile([P, NC_OUT], fp32, tag="outsb")
                nc.scalar.activation(
                    out=out_sb[:], in_=pout[:],
                    func=mybir.ActivationFunctionType.Copy,
                )
                nc.sync.dma_start(out[r0:r0 + P, :], out_sb[:])

    nc.compile()
    return nc


def _prep_weights(W1, b1, W2, b2):
    bf16 = ml_dtypes.bfloat16
    w1t = (
        W1.T.reshape(KCH, P, H1).transpose(1, 0, 2).reshape(P, KCH * H1)
        .astype(bf16)
    )
    w2t = (
        W2.T.reshape(HCH, P, NC_OUT).transpose(1, 0, 2).reshape(P, HCH * NC_OUT)
        .astype(bf16)
    )
    b1t = np.ascontiguousarray(b1.reshape(HCH, P).T)
    b2c = np.ascontiguousarray(b2.reshape(NC_OUT, 1))
    return w1t, w2t, b1t, b2c


_NC_CACHE = {}


def kernel(sim_stance_emb, nli_stance_emb, sim_body_emb, nli_body_emb,
           W1, b1, W2, b2, _trace=False, _tmpdir=None):
    from concourse.bass_utils import run_bass_kernel_spmd

    sim_stance_emb = np.asarray(sim_stance_emb, dtype=np.float32)
    nli_stance_emb = np.asarray(nli_stance_emb, dtype=np.float32)
    sim_body_emb = np.asarray(sim_body_emb, dtype=np.float32)
    nli_body_emb = np.asarray(nli_body_emb, dtype=np.float32)
    w1t, w2t, b1t, b2c = _prep_weights(
        np.asarray(W1, np.float32), np.asarray(b1, np.float32),
        np.asarray(W2, np.float32), np.asarray(b2, np.float32))

    if "nc" not in _NC_CACHE:
        _NC_CACHE["nc"] = build(BL)
    nc = _NC_CACHE["nc"]

    in_maps = []
    for i in range(N_CORES):
        r = slice(i * BL, (i + 1) * BL)
        in_maps.append({
            "sim_stance": sim_stance_emb[r],
            "nli_stance": nli_stance_emb[r],
            "sim_body": np.ascontiguousarray(
                sim_body_emb[r].reshape(BL, N * D)),
            "nli_body": np.ascontiguousarray(
                nli_body_emb[r].reshape(BL * N, D)),
            "w1t": w1t, "w2t": w2t, "b1t": b1t, "b2c": b2c,
        })

    res = run_bass_kernel_spmd(
        nc, in_maps, core_ids=list(range(N_CORES)),
        trace=_trace, tmpdir=_tmpdir,
    )
    out = np.concatenate([res.results[i]["out"] for i in range(N_CORES)], axis=0)
    if _trace:
        kernel.last_exec_time_ns = res.exec_time_ns
    return out


# revision 3
# speedup vs baseline: 1.0052x; 1.0052x over previous
"""Trainium2 Bass kernel for the AgreemFlat top-k masking model.

Reference computation (per batch row b):
    sims[b, n]  = <sim_stance_emb[b], sim_body_emb[b, n]>         n in [0, 64)
    top5        = top_k(sims[b], 5).indices                        (descending)
    xx[b]       = [nli_stance_emb[b] | nli_body_emb[b, top5].ravel()]   (4608)
    out[b]      = relu(xx @ W1.T + b1) @ W2.T + b2                 (4 classes)

Sharding: pure data-parallel over B=4096 -> 8 cores x 512 rows.
Weights replicated; W1^T / W2^T / b1 are pre-arranged host-side into
partition-major layouts so the device only does straight DMA loads.

Per-core dataflow (chunks of 128 batch rows on partitions):
  DMA sim_body stream -> DVE tensor_tensor_reduce (fp32 dot products)
  -> vector.max / max_index (top-8, take 5) -> indirect DMA gather
  -> PE transpose (+ ACT cast to bf16) -> PE matmul FC1 (bf16, fp32 psum)
  -> ACT fused bias+relu -> PE matmul FC2 -> bias add -> PE transpose -> DMA out.
"""

import numpy as np
import ml_dtypes

import concourse.bass as bass
import concourse.mybir as mybir
import concourse.tile as tile
from concourse import bacc
from concourse.masks import make_identity

P = 128
N = 64            # body sentences per row
D = 768           # embedding dim (sim and nli)
KK = 5            # top-k
H1 = 1024         # fc1 output dim
NC_OUT = 4        # classes
FAN1 = (KK + 1) * D          # 4608 = fc1 fan-in
KCH = FAN1 // P              # 36 contraction chunks
HCH = H1 // P                # 8 fc1-output chunks
N_CORES = 8
B_FULL = 4096
BL = B_FULL // N_CORES       # 512 rows per core
NSUB = 8                     # body n-sentences per streamed subtile


def build(bl=BL):
    chunks = bl // P
    nsubs = N // NSUB
    fp32 = mybir.dt.float32
    bf16 = mybir.dt.bfloat16
    u32 = mybir.dt.uint32

    nc = bacc.Bacc("TRN2", target_bir_lowering=False)

    sim_stance = nc.dram_tensor("sim_stance", [bl, D], fp32, kind="ExternalInput")
    nli_stance = nc.dram_tensor("nli_stance", [bl, D], fp32, kind="ExternalInput")
    sim_body = nc.dram_tensor("sim_body", [bl, N * D], fp32, kind="ExternalInput")
    nli_body = nc.dram_tensor("nli_body", [bl * N, D], fp32, kind="ExternalInput")
    w1t = nc.dram_tensor("w1t", [P, KCH * H1], bf16, kind="ExternalInput")
    w2t = nc.dram_tensor("w2t", [P, HCH * NC_OUT], bf16, kind="ExternalInput")
    b1t = nc.dram_tensor("b1t", [P, HCH], fp32, kind="ExternalInput")
    b2c = nc.dram_tensor("b2c", [NC_OUT, 1], fp32, kind="ExternalInput")
    out = nc.dram_tensor("out", [bl, NC_OUT], fp32, kind="ExternalOutput")

    with tile.TileContext(nc) as tc:
        with (
            tc.tile_pool(name="wpool", bufs=1) as wpool,
            tc.tile_pool(name="bodyp", bufs=2) as bodyp,
            tc.tile_pool(name="stancep", bufs=2) as stancep,
            tc.tile_pool(name="simsp", bufs=2) as simsp,
            tc.tile_pool(name="topkp", bufs=2) as topkp,
            tc.tile_pool(name="gp", bufs=2) as gp,
            tc.tile_pool(name="xxtp", bufs=2) as xxtp,
            tc.tile_pool(name="yp", bufs=2) as yp,
            tc.tile_pool(name="op", bufs=2) as op,
            tc.tile_pool(name="pt", bufs=2, space="PSUM") as pt_pool,
            tc.tile_pool(name="py", bufs=2, space="PSUM") as py_pool,
            tc.tile_pool(name="po", bufs=2, space="PSUM") as po_pool,
        ):
            # ---- setup: weights, biases, identity ----
            w1t_sb = wpool.tile([P, KCH * H1], bf16)
            nc.scalar.dma_start(w1t_sb[:], w1t[:, :])
            w2t_sb = wpool.tile([P, HCH * NC_OUT], bf16)
            nc.scalar.dma_start(w2t_sb[:], w2t[:, :])
            b1t_sb = wpool.tile([P, HCH], fp32)
            nc.scalar.dma_start(b1t_sb[:], b1t[:, :])
            b2_sb = wpool.tile([NC_OUT, 1], fp32)
            nc.scalar.dma_start(b2_sb[:], b2c[:, :])
            ident = wpool.tile([P, P], fp32)
            make_identity(nc, ident[:])

            for ch in range(chunks):
                r0 = ch * P
                # ---- stance tiles for this chunk ----
                sstance = stancep.tile([P, D], fp32, tag="sstance")
                nc.sync.dma_start(sstance[:], sim_stance[r0:r0 + P, :])
                nstance = stancep.tile([P, D], fp32, tag="nstance")
                nc.sync.dma_start(nstance[:], nli_stance[r0:r0 + P, :])

                # ---- sims: stream body, fused multiply+reduce on DVE ----
                sims = simsp.tile([P, N], fp32)
                for s in range(nsubs):
                    body = bodyp.tile([P, NSUB * D], fp32, tag="body")
                    nc.sync.dma_start(
                        body[:],
                        sim_body[r0:r0 + P, s * NSUB * D:(s + 1) * NSUB * D],
                    )
                    for j in range(NSUB):
                        nrow = s * NSUB + j
                        seg = body[:, j * D:(j + 1) * D]
                        nc.vector.scalar_tensor_tensor(
                            out=seg,
                            in0=seg,
                            scalar=1.0,
                            in1=sstance[:],
                            op0=mybir.AluOpType.mult,
                            op1=mybir.AluOpType.mult,
                            accum_out=sims[:, nrow:nrow + 1],
                        )

                # ---- top-5 of 64 (descending, matches jax.lax.top_k) ----
                max8 = topkp.tile([P, 8], fp32, tag="max8")
                nc.vector.max(out=max8[:], in_=sims[:])
                idx8 = topkp.tile([P, 8], u32, tag="idx8")
                nc.vector.max_index(out=idx8[:], in_max=max8[:], in_values=sims[:])

                # global row index into nli_body rows: (r0 + p) * 64 + idx
                rowbase = topkp.tile([P, 8], u32, tag="rowbase")
                nc.gpsimd.iota(
                    rowbase[:], [[0, 8]], base=r0 * N, channel_multiplier=N,
                )
                rows = topkp.tile([P, 8], u32, tag="rows")
                nc.vector.tensor_tensor(
                    out=rows[:], in0=idx8[:], in1=rowbase[:],
                    op=mybir.AluOpType.add,
                )

                # ---- gather top-5 nli body embeddings: [128, 5*768] ----
                xg = gp.tile([P, KK * D], fp32)
                for k in range(KK):
                    nc.gpsimd.indirect_dma_start(
                        out=xg[:, k * D:(k + 1) * D],
                        out_offset=None,
                        in_=nli_body[:, :],
                        in_offset=bass.IndirectOffsetOnAxis(
                            ap=rows[:, k:k + 1], axis=0
                        ),
                    )

                # ---- xx^T in bf16 via PE transpose + ACT cast ----
                # xx = [nli_stance | gathered] : [128, 4608]
                xxt = xxtp.tile([P, FAN1], bf16)
                for t in range(KCH):
                    if t < D // P:
                        src = nstance[:, t * P:(t + 1) * P]
                    else:
                        g = t - D // P
                        src = xg[:, g * P:(g + 1) * P]
                    ptile = pt_pool.tile([P, P], fp32, tag="pt")
                    nc.tensor.transpose(out=ptile[:], in_=src, identity=ident[:])
                    nc.scalar.activation(
                        out=xxt[:, t * P:(t + 1) * P], in_=ptile[:],
                        func=mybir.ActivationFunctionType.Copy,
                    )

                # ---- FC1 (bf16): y^T[h, b] = W1 @ xx^T, fused bias+relu ----
                yt = yp.tile([P, H1], bf16)
                for m in range(HCH):
                    py = py_pool.tile([P, P], fp32, tag="py")
                    for c in range(KCH):
                        nc.tensor.matmul(
                            out=py[:],
                            lhsT=w1t_sb[:, c * H1 + m * P:c * H1 + (m + 1) * P],
                            rhs=xxt[:, c * P:(c + 1) * P],
                            start=(c == 0),
                            stop=(c == KCH - 1),
                        )
                    nc.scalar.activation(
                        out=yt[:, m * P:(m + 1) * P], in_=py[:],
                        func=mybir.ActivationFunctionType.Relu,
                        bias=b1t_sb[:, m:m + 1],
                    )

                # ---- FC2: o^T[4, b] = W2 @ y^T ----
                po = po_pool.tile([NC_OUT, P], fp32, tag="po")
                for m in range(HCH):
                    nc.tensor.matmul(
                        out=po[:],
                        lhsT=w2t_sb[:, m * NC_OUT:(m + 1) * NC_OUT],
                        rhs=yt[:, m * P:(m + 1) * P],
                        start=(m == 0),
                        stop=(m == HCH - 1),
                    )
                osb = op.tile([NC_OUT, P], fp32, tag="osb")
                nc.vector.tensor_tensor(
                    out=osb[:], in0=po[:],
                    in1=b2_sb[:].to_broadcast([NC_OUT, P]),
                    op=mybir.AluOpType.add,
                )

                # ---- transpose [4, 128] -> [128, 4] and store ----
                pout = pt_pool.tile([P, NC_OUT], fp32, tag="pout")
                nc.tensor.transpose(
                    out=pout[:], in_=osb[:], identity=ident[:NC_OUT, :NC_OUT]
                )
                out_sb = op.tile([P, NC_OUT], fp32, tag="outsb")
                nc.scalar.activation(
                    out=out_sb[:], in_=pout[:],
                    func=mybir.ActivationFunctionType.Copy,
                )
                nc.scalar.dma_start(out[r0:r0 + P, :], out_sb[:])

    nc.compile()
    return nc


def _prep_weights(W1, b1, W2, b2):
    bf16 = ml_dtypes.bfloat16
    w1t = (
        W1.T.reshape(KCH, P, H1).transpose(1, 0, 2).reshape(P, KCH * H1)
        .astype(bf16)
    )
    w2t = (
        W2.T.reshape(HCH, P, NC_OUT).transpose(1, 0, 2).reshape(P, HCH * NC_OUT)
        .astype(bf16)
    )
    b1t = np.ascontiguousarray(b1.reshape(HCH, P).T)
    b2c = np.ascontiguousarray(b2.reshape(NC_OUT, 1))
    return w1t, w2t, b1t, b2c


_NC_CACHE = {}


def kernel(sim_stance_emb, nli_stance_emb, sim_body_emb, nli_body_emb,
           W1, b1, W2, b2, _trace=False, _tmpdir=None):
    from concourse.bass_utils import run_bass_kernel_spmd

    sim_stance_emb = np.asarray(sim_stance_emb, dtype=np.float32)
    nli_stance_emb = np.asarray(nli_stance_emb, dtype=np.float32)
    sim_body_emb = np.asarray(sim_body_emb, dtype=np.float32)
    nli_body_emb = np.asarray(nli_body_emb, dtype=np.float32)
    w1t, w2t, b1t, b2c = _prep_weights(
        np.asarray(W1, np.float32), np.asarray(b1, np.float32),
        np.asarray(W2, np.float32), np.asarray(b2, np.float32))

    if "nc" not in _NC_CACHE:
        _NC_CACHE["nc"] = build(BL)
    nc = _NC_CACHE["nc"]

    in_maps = []
    for i in range(N_CORES):
        r = slice(i * BL, (i + 1) * BL)
        in_maps.append({
            "sim_stance": sim_stance_emb[r],
            "nli_stance": nli_stance_emb[r],
            "sim_body": np.ascontiguousarray(
                sim_body_emb[r].reshape(BL, N * D)),
            "nli_body": np.ascontiguousarray(
                nli_body_emb[r].reshape(BL * N, D)),
            "w1t": w1t, "w2t": w2t, "b1t": b1t, "b2c": b2c,
        })

    res = run_bass_kernel_spmd(
        nc, in_maps, core_ids=list(range(N_CORES)),
        trace=_trace, tmpdir=_tmpdir,
    )
    out = np.concatenate([res.results[i]["out"] for i in range(N_CORES)], axis=0)
    if _trace:
        kernel.last_exec_time_ns = res.exec_time_ns
    return out


# revision 4
# speedup vs baseline: 1.1245x; 1.1187x over previous
"""Trainium2 Bass kernel for the AgreemFlat top-k masking model.

Reference computation (per batch row b):
    sims[b, n]  = <sim_stance_emb[b], sim_body_emb[b, n]>         n in [0, 64)
    top5        = top_k(sims[b], 5).indices                        (descending)
    xx[b]       = [nli_stance_emb[b] | nli_body_emb[b, top5].ravel()]   (4608)
    out[b]      = relu(xx @ W1.T + b1) @ W2.T + b2                 (4 classes)

Sharding: pure data-parallel over B=4096 -> 8 cores x 512 rows.
Weights replicated; W1^T / W2^T / b1 are pre-arranged host-side into
partition-major layouts so the device only does straight DMA loads.

Per-core dataflow (chunks of 128 batch rows on partitions):
  DMA sim_body stream -> DVE tensor_tensor_reduce (fp32 dot products)
  -> vector.max / max_index (top-8, take 5) -> indirect DMA gather
  -> PE transpose (+ ACT cast to bf16) -> PE matmul FC1 (bf16, fp32 psum)
  -> ACT fused bias+relu -> PE matmul FC2 -> bias add -> PE transpose -> DMA out.
"""

import numpy as np
import ml_dtypes

import concourse.bass as bass
import concourse.mybir as mybir
import concourse.tile as tile
from concourse import bacc
from concourse.masks import make_identity

P = 128
N = 64            # body sentences per row
D = 768           # embedding dim (sim and nli)
KK = 5            # top-k
H1 = 1024         # fc1 output dim
NC_OUT = 4        # classes
FAN1 = (KK + 1) * D          # 4608 = fc1 fan-in
KCH = FAN1 // P              # 36 contraction chunks
HCH = H1 // P                # 8 fc1-output chunks
N_CORES = 8
B_FULL = 4096
BL = B_FULL // N_CORES       # 512 rows per core
NSUB = 8                     # body n-sentences per streamed subtile


def build(bl=BL):
    chunks = bl // P
    nsubs = N // NSUB
    fp32 = mybir.dt.float32
    bf16 = mybir.dt.bfloat16
    u32 = mybir.dt.uint32

    nc = bacc.Bacc("TRN2", target_bir_lowering=False)

    sim_stance = nc.dram_tensor("sim_stance", [bl, D], fp32, kind="ExternalInput")
    nli_stance = nc.dram_tensor("nli_stance", [bl, D], bf16, kind="ExternalInput")
    sim_body = nc.dram_tensor("sim_body", [bl, N * D], fp32, kind="ExternalInput")
    nli_body = nc.dram_tensor("nli_body", [bl * N, D], bf16, kind="ExternalInput")
    w1t = nc.dram_tensor("w1t", [P, KCH * H1], bf16, kind="ExternalInput")
    w2t = nc.dram_tensor("w2t", [P, HCH * NC_OUT], bf16, kind="ExternalInput")
    b1t = nc.dram_tensor("b1t", [P, HCH], fp32, kind="ExternalInput")
    b2c = nc.dram_tensor("b2c", [NC_OUT, 1], fp32, kind="ExternalInput")
    out = nc.dram_tensor("out", [bl, NC_OUT], fp32, kind="ExternalOutput")

    with tile.TileContext(nc) as tc:
        with (
            tc.tile_pool(name="wpool", bufs=1) as wpool,
            tc.tile_pool(name="bodyp", bufs=2) as bodyp,
            tc.tile_pool(name="stancep", bufs=2) as stancep,
            tc.tile_pool(name="simsp", bufs=2) as simsp,
            tc.tile_pool(name="topkp", bufs=2) as topkp,
            tc.tile_pool(name="gp", bufs=2) as gp,
            tc.tile_pool(name="xxtp", bufs=2) as xxtp,
            tc.tile_pool(name="yp", bufs=2) as yp,
            tc.tile_pool(name="op", bufs=2) as op,
            tc.tile_pool(name="pt", bufs=2, space="PSUM") as pt_pool,
            tc.tile_pool(name="py", bufs=2, space="PSUM") as py_pool,
            tc.tile_pool(name="po", bufs=2, space="PSUM") as po_pool,
        ):
            # ---- setup: weights, biases, identity ----
            w1t_sb = wpool.tile([P, KCH * H1], bf16)
            nc.scalar.dma_start(w1t_sb[:], w1t[:, :])
            w2t_sb = wpool.tile([P, HCH * NC_OUT], bf16)
            nc.scalar.dma_start(w2t_sb[:], w2t[:, :])
            b1t_sb = wpool.tile([P, HCH], fp32)
            nc.scalar.dma_start(b1t_sb[:], b1t[:, :])
            b2_sb = wpool.tile([NC_OUT, 1], fp32)
            nc.scalar.dma_start(b2_sb[:], b2c[:, :])
            ident = wpool.tile([P, P], fp32)
            make_identity(nc, ident[:])
            identb = wpool.tile([P, P], bf16)
            make_identity(nc, identb[:])

            for ch in range(chunks):
                r0 = ch * P
                # ---- stance tiles for this chunk ----
                sstance = stancep.tile([P, D], fp32, tag="sstance")
                nc.sync.dma_start(sstance[:], sim_stance[r0:r0 + P, :])
                nstance = stancep.tile([P, D], bf16, tag="nstance")
                nc.sync.dma_start(nstance[:], nli_stance[r0:r0 + P, :])

                # ---- sims: stream body, fused multiply+reduce on DVE ----
                sims = simsp.tile([P, N], fp32)
                for s in range(nsubs):
                    body = bodyp.tile([P, NSUB * D], fp32, tag="body")
                    nc.sync.dma_start(
                        body[:],
                        sim_body[r0:r0 + P, s * NSUB * D:(s + 1) * NSUB * D],
                    )
                    for j in range(NSUB):
                        nrow = s * NSUB + j
                        seg = body[:, j * D:(j + 1) * D]
                        nc.vector.scalar_tensor_tensor(
                            out=seg,
                            in0=seg,
                            scalar=1.0,
                            in1=sstance[:],
                            op0=mybir.AluOpType.mult,
                            op1=mybir.AluOpType.mult,
                            accum_out=sims[:, nrow:nrow + 1],
                        )

                # ---- top-5 of 64 (descending, matches jax.lax.top_k) ----
                max8 = topkp.tile([P, 8], fp32, tag="max8")
                nc.vector.max(out=max8[:], in_=sims[:])
                idx8 = topkp.tile([P, 8], u32, tag="idx8")
                nc.vector.max_index(out=idx8[:], in_max=max8[:], in_values=sims[:])

                # global row index into nli_body rows: (r0 + p) * 64 + idx
                rowbase = topkp.tile([P, 8], u32, tag="rowbase")
                nc.gpsimd.iota(
                    rowbase[:], [[0, 8]], base=r0 * N, channel_multiplier=N,
                )
                rows = topkp.tile([P, 8], u32, tag="rows")
                nc.vector.tensor_tensor(
                    out=rows[:], in0=idx8[:], in1=rowbase[:],
                    op=mybir.AluOpType.add,
                )

                # ---- gather top-5 nli body embeddings: [128, 5*768] ----
                xg = gp.tile([P, KK * D], bf16)
                for k in range(KK):
                    nc.gpsimd.indirect_dma_start(
                        out=xg[:, k * D:(k + 1) * D],
                        out_offset=None,
                        in_=nli_body[:, :],
                        in_offset=bass.IndirectOffsetOnAxis(
                            ap=rows[:, k:k + 1], axis=0
                        ),
                    )

                # ---- xx^T in bf16 via PE transpose + ACT cast ----
                # xx = [nli_stance | gathered] : [128, 4608]
                xxt = xxtp.tile([P, FAN1], bf16)
                for t in range(KCH):
                    if t < D // P:
                        src = nstance[:, t * P:(t + 1) * P]
                    else:
                        g = t - D // P
                        src = xg[:, g * P:(g + 1) * P]
                    ptile = pt_pool.tile([P, P], bf16, tag="pt")
                    nc.tensor.transpose(
                        out=ptile[:], in_=src, identity=identb[:])
                    nc.scalar.activation(
                        out=xxt[:, t * P:(t + 1) * P], in_=ptile[:],
                        func=mybir.ActivationFunctionType.Copy,
                    )

                # ---- FC1 (bf16): y^T[h, b] = W1 @ xx^T, fused bias+relu ----
                yt = yp.tile([P, H1], bf16)
                for m in range(HCH):
                    py = py_pool.tile([P, P], fp32, tag="py")
                    for c in range(KCH):
                        nc.tensor.matmul(
                            out=py[:],
                            lhsT=w1t_sb[:, c * H1 + m * P:c * H1 + (m + 1) * P],
                            rhs=xxt[:, c * P:(c + 1) * P],
                            start=(c == 0),
                            stop=(c == KCH - 1),
                        )
                    nc.scalar.activation(
                        out=yt[:, m * P:(m + 1) * P], in_=py[:],
                        func=mybir.ActivationFunctionType.Relu,
                        bias=b1t_sb[:, m:m + 1],
                    )

                # ---- FC2: o^T[4, b] = W2 @ y^T ----
                po = po_pool.tile([NC_OUT, P], fp32, tag="po")
                for m in range(HCH):
                    nc.tensor.matmul(
                        out=po[:],
                        lhsT=w2t_sb[:, m * NC_OUT:(m + 1) * NC_OUT],
                        rhs=yt[:, m * P:(m + 1) * P],
                        start=(m == 0),
                        stop=(m == HCH - 1),
                    )
                osb = op.tile([NC_OUT, P], fp32, tag="osb")
                nc.scalar.activation(
                    out=osb[:], in_=po[:],
                    func=mybir.ActivationFunctionType.Identity,
                    bias=b2_sb[:, 0:1],
                )

                # ---- transpose [4, 128] -> [128, 4] and store ----
                pout = pt_pool.tile([P, NC_OUT], fp32, tag="pout")
                nc.tensor.transpose(
                    out=pout[:], in_=osb[:], identity=ident[:NC_OUT, :NC_OUT]
                )
                out_sb = op.tile([P, NC_OUT], fp32, tag="outsb")
                nc.scalar.activation(
                    out=out_sb[:], in_=pout[:],
                    func=mybir.ActivationFunctionType.Copy,
                )
                nc.scalar.dma_start(out[r0:r0 + P, :], out_sb[:])

    nc.compile()
    return nc


def _prep_weights(W1, b1, W2, b2):
    bf16 = ml_dtypes.bfloat16
    w1t = (
        W1.T.reshape(KCH, P, H1).transpose(1, 0, 2).reshape(P, KCH * H1)
        .astype(bf16)
    )
    w2t = (
        W2.T.reshape(HCH, P, NC_OUT).transpose(1, 0, 2).reshape(P, HCH * NC_OUT)
        .astype(bf16)
    )
    b1t = np.ascontiguousarray(b1.reshape(HCH, P).T)
    b2c = np.ascontiguousarray(b2.reshape(NC_OUT, 1))
    return w1t, w2t, b1t, b2c


_NC_CACHE = {}


def kernel(sim_stance_emb, nli_stance_emb, sim_body_emb, nli_body_emb,
           W1, b1, W2, b2, _trace=False, _tmpdir=None):
    from concourse.bass_utils import run_bass_kernel_spmd

    sim_stance_emb = np.asarray(sim_stance_emb, dtype=np.float32)
    nli_stance_emb = np.asarray(nli_stance_emb, dtype=np.float32).astype(
        ml_dtypes.bfloat16)
    sim_body_emb = np.asarray(sim_body_emb, dtype=np.float32)
    nli_body_emb = np.asarray(nli_body_emb, dtype=np.float32).astype(
        ml_dtypes.bfloat16)
    w1t, w2t, b1t, b2c = _prep_weights(
        np.asarray(W1, np.float32), np.asarray(b1, np.float32),
        np.asarray(W2, np.float32), np.asarray(b2, np.float32))

    if "nc" not in _NC_CACHE:
        _NC_CACHE["nc"] = build(BL)
    nc = _NC_CACHE["nc"]

    in_maps = []
    for i in range(N_CORES):
        r = slice(i * BL, (i + 1) * BL)
        in_maps.append({
            "sim_stance": sim_stance_emb[r],
            "nli_stance": nli_stance_emb[r],
            "sim_body": np.ascontiguousarray(
                sim_body_emb[r].reshape(BL, N * D)),
            "nli_body": np.ascontiguousarray(
                nli_body_emb[r].reshape(BL * N, D)),
            "w1t": w1t, "w2t": w2t, "b1t": b1t, "b2c": b2c,
        })

    res = run_bass_kernel_spmd(
        nc, in_maps, core_ids=list(range(N_CORES)),
        trace=_trace, tmpdir=_tmpdir,
    )
    out = np.concatenate([res.results[i]["out"] for i in range(N_CORES)], axis=0)
    if _trace:
        kernel.last_exec_time_ns = res.exec_time_ns
    return out


# revision 5
# speedup vs baseline: 1.1830x; 1.0520x over previous
"""Trainium2 Bass kernel for the AgreemFlat top-k masking model.

Reference computation (per batch row b):
    sims[b, n]  = <sim_stance_emb[b], sim_body_emb[b, n]>         n in [0, 64)
    top5        = top_k(sims[b], 5).indices                        (descending)
    xx[b]       = [nli_stance_emb[b] | nli_body_emb[b, top5].ravel()]   (4608)
    out[b]      = relu(xx @ W1.T + b1) @ W2.T + b2                 (4 classes)

Sharding: pure data-parallel over B=4096 -> 8 cores x 512 rows.
Weights replicated; W1^T / W2^T / b1 are pre-arranged host-side into
partition-major layouts so the device only does straight DMA loads.

Per-core dataflow (chunks of 128 batch rows on partitions):
  DMA sim_body stream -> DVE tensor_tensor_reduce (fp32 dot products)
  -> vector.max / max_index (top-8, take 5) -> indirect DMA gather
  -> PE transpose (+ ACT cast to bf16) -> PE matmul FC1 (bf16, fp32 psum)
  -> ACT fused bias+relu -> PE matmul FC2 -> bias add -> PE transpose -> DMA out.
"""

import numpy as np
import ml_dtypes

import concourse.bass as bass
import concourse.mybir as mybir
import concourse.tile as tile
from concourse import bacc
from concourse.masks import make_identity

P = 128
N = 64            # body sentences per row
D = 768           # embedding dim (sim and nli)
KK = 5            # top-k
H1 = 1024         # fc1 output dim
NC_OUT = 4        # classes
FAN1 = (KK + 1) * D          # 4608 = fc1 fan-in
KCH = FAN1 // P              # 36 contraction chunks
HCH = H1 // P                # 8 fc1-output chunks
N_CORES = 8
B_FULL = 4096
BL = B_FULL // N_CORES       # 512 rows per core
NSUB = 8                     # body n-sentences per streamed subtile


def build(bl=BL):
    chunks = bl // P
    nsubs = N // NSUB
    fp32 = mybir.dt.float32
    bf16 = mybir.dt.bfloat16
    u32 = mybir.dt.uint32

    nc = bacc.Bacc("TRN2", target_bir_lowering=False)

    sim_stance = nc.dram_tensor("sim_stance", [bl, D], fp32, kind="ExternalInput")
    nli_stance = nc.dram_tensor("nli_stance", [bl, D], bf16, kind="ExternalInput")
    sim_body = nc.dram_tensor("sim_body", [bl, N * D], fp32, kind="ExternalInput")
    nli_body = nc.dram_tensor("nli_body", [bl * N, D], bf16, kind="ExternalInput")
    w1t = nc.dram_tensor("w1t", [P, KCH * H1], bf16, kind="ExternalInput")
    w2t = nc.dram_tensor("w2t", [P, HCH * NC_OUT], bf16, kind="ExternalInput")
    b1t = nc.dram_tensor("b1t", [P, HCH], fp32, kind="ExternalInput")
    b2c = nc.dram_tensor("b2c", [NC_OUT, 1], fp32, kind="ExternalInput")
    out = nc.dram_tensor("out", [bl, NC_OUT], fp32, kind="ExternalOutput")

    with tile.TileContext(nc) as tc:
        with (
            tc.tile_pool(name="wpool", bufs=1) as wpool,
            tc.tile_pool(name="bodyp", bufs=2) as bodyp,
            tc.tile_pool(name="stancep", bufs=2) as stancep,
            tc.tile_pool(name="simsp", bufs=2) as simsp,
            tc.tile_pool(name="topkp", bufs=2) as topkp,
            tc.tile_pool(name="gp", bufs=2) as gp,
            tc.tile_pool(name="xxtp", bufs=2) as xxtp,
            tc.tile_pool(name="yp", bufs=2) as yp,
            tc.tile_pool(name="op", bufs=2) as op,
            tc.tile_pool(name="pt", bufs=2, space="PSUM") as pt_pool,
            tc.tile_pool(name="py", bufs=2, space="PSUM") as py_pool,
            tc.tile_pool(name="po", bufs=2, space="PSUM") as po_pool,
        ):
            # ---- setup: weights, biases, identity ----
            w1t_sb = wpool.tile([P, KCH * H1], bf16)
            nc.gpsimd.dma_start(w1t_sb[:], w1t[:, :])
            w2t_sb = wpool.tile([P, HCH * NC_OUT], bf16)
            nc.gpsimd.dma_start(w2t_sb[:], w2t[:, :])
            b1t_sb = wpool.tile([P, HCH], fp32)
            nc.gpsimd.dma_start(b1t_sb[:], b1t[:, :])
            b2_sb = wpool.tile([NC_OUT, 1], fp32)
            nc.gpsimd.dma_start(b2_sb[:], b2c[:, :])
            ident = wpool.tile([P, P], fp32)
            make_identity(nc, ident[:])
            identb = wpool.tile([P, P], bf16)
            make_identity(nc, identb[:])

            for ch in range(chunks):
                r0 = ch * P
                # ---- stance tiles for this chunk ----
                sstance = stancep.tile([P, D], fp32, tag="sstance")
                nc.sync.dma_start(sstance[:], sim_stance[r0:r0 + P, :])
                nstance = stancep.tile([P, D], bf16, tag="nstance")
                nc.sync.dma_start(nstance[:], nli_stance[r0:r0 + P, :])

                # ---- sims: stream body, fused multiply+reduce on DVE ----
                sims = simsp.tile([P, N], fp32)
                for s in range(nsubs):
                    body = bodyp.tile([P, NSUB * D], fp32, tag="body")
                    nc.sync.dma_start(
                        body[:],
                        sim_body[r0:r0 + P, s * NSUB * D:(s + 1) * NSUB * D],
                    )
                    for j in range(NSUB):
                        nrow = s * NSUB + j
                        seg = body[:, j * D:(j + 1) * D]
                        nc.vector.scalar_tensor_tensor(
                            out=seg,
                            in0=seg,
                            scalar=1.0,
                            in1=sstance[:],
                            op0=mybir.AluOpType.mult,
                            op1=mybir.AluOpType.mult,
                            accum_out=sims[:, nrow:nrow + 1],
                        )

                # ---- top-5 of 64 (descending, matches jax.lax.top_k) ----
                max8 = topkp.tile([P, 8], fp32, tag="max8")
                nc.vector.max(out=max8[:], in_=sims[:])
                idx8 = topkp.tile([P, 8], u32, tag="idx8")
                nc.vector.max_index(out=idx8[:], in_max=max8[:], in_values=sims[:])

                # global row index into nli_body rows: (r0 + p) * 64 + idx
                rowbase = topkp.tile([P, 8], u32, tag="rowbase")
                nc.gpsimd.iota(
                    rowbase[:], [[0, 8]], base=r0 * N, channel_multiplier=N,
                )
                rows = topkp.tile([P, 8], u32, tag="rows")
                nc.vector.tensor_tensor(
                    out=rows[:], in0=idx8[:], in1=rowbase[:],
                    op=mybir.AluOpType.add,
                )

                # ---- gather top-5 nli body embeddings: [128, 5*768] ----
                xg = gp.tile([P, KK * D], bf16)
                for k in range(KK):
                    nc.gpsimd.indirect_dma_start(
                        out=xg[:, k * D:(k + 1) * D],
                        out_offset=None,
                        in_=nli_body[:, :],
                        in_offset=bass.IndirectOffsetOnAxis(
                            ap=rows[:, k:k + 1], axis=0
                        ),
                    )

                # ---- xx^T in bf16 via PE transpose + ACT cast ----
                # xx = [nli_stance | gathered] : [128, 4608]
                xxt = xxtp.tile([P, FAN1], bf16)
                for t in range(KCH):
                    if t < D // P:
                        src = nstance[:, t * P:(t + 1) * P]
                    else:
                        g = t - D // P
                        src = xg[:, g * P:(g + 1) * P]
                    ptile = pt_pool.tile([P, P], bf16, tag="pt")
                    nc.tensor.transpose(
                        out=ptile[:], in_=src, identity=identb[:])
                    nc.scalar.activation(
                        out=xxt[:, t * P:(t + 1) * P], in_=ptile[:],
                        func=mybir.ActivationFunctionType.Copy,
                    )

                # ---- FC1 (bf16): y^T[h, b] = W1 @ xx^T, fused bias+relu ----
                yt = yp.tile([P, H1], bf16)
                for m in range(HCH):
                    py = py_pool.tile([P, P], fp32, tag="py")
                    for c in range(KCH):
                        nc.tensor.matmul(
                            out=py[:],
                            lhsT=w1t_sb[:, c * H1 + m * P:c * H1 + (m + 1) * P],
                            rhs=xxt[:, c * P:(c + 1) * P],
                            start=(c == 0),
                            stop=(c == KCH - 1),
                        )
                    nc.scalar.activation(
                        out=yt[:, m * P:(m + 1) * P], in_=py[:],
                        func=mybir.ActivationFunctionType.Relu,
                        bias=b1t_sb[:, m:m + 1],
                    )

                # ---- FC2: o^T[4, b] = W2 @ y^T ----
                po = po_pool.tile([NC_OUT, P], fp32, tag="po")
                for m in range(HCH):
                    nc.tensor.matmul(
                        out=po[:],
                        lhsT=w2t_sb[:, m * NC_OUT:(m + 1) * NC_OUT],
                        rhs=yt[:, m * P:(m + 1) * P],
                        start=(m == 0),
                        stop=(m == HCH - 1),
                    )
                osb = op.tile([NC_OUT, P], fp32, tag="osb")
                nc.scalar.activation(
                    out=osb[:], in_=po[:],
                    func=mybir.ActivationFunctionType.Identity,
                    bias=b2_sb[:, 0:1],
                )

                # ---- transpose [4, 128] -> [128, 4] and store ----
                pout = pt_pool.tile([P, NC_OUT], fp32, tag="pout")
                nc.tensor.transpose(
                    out=pout[:], in_=osb[:], identity=ident[:NC_OUT, :NC_OUT]
                )
                out_sb = op.tile([P, NC_OUT], fp32, tag="outsb")
                nc.scalar.activation(
                    out=out_sb[:], in_=pout[:],
                    func=mybir.ActivationFunctionType.Copy,
                )
                nc.scalar.dma_start(out[r0:r0 + P, :], out_sb[:])

    nc.compile()
    return nc


def _prep_weights(W1, b1, W2, b2):
    bf16 = ml_dtypes.bfloat16
    w1t = (
        W1.T.reshape(KCH, P, H1).transpose(1, 0, 2).reshape(P, KCH * H1)
        .astype(bf16)
    )
    w2t = (
        W2.T.reshape(HCH, P, NC_OUT).transpose(1, 0, 2).reshape(P, HCH * NC_OUT)
        .astype(bf16)
    )
    b1t = np.ascontiguousarray(b1.reshape(HCH, P).T)
    b2c = np.ascontiguousarray(b2.reshape(NC_OUT, 1))
    return w1t, w2t, b1t, b2c


_NC_CACHE = {}


def kernel(sim_stance_emb, nli_stance_emb, sim_body_emb, nli_body_emb,
           W1, b1, W2, b2, _trace=False, _tmpdir=None):
    from concourse.bass_utils import run_bass_kernel_spmd

    sim_stance_emb = np.asarray(sim_stance_emb, dtype=np.float32)
    nli_stance_emb = np.asarray(nli_stance_emb, dtype=np.float32).astype(
        ml_dtypes.bfloat16)
    sim_body_emb = np.asarray(sim_body_emb, dtype=np.float32)
    nli_body_emb = np.asarray(nli_body_emb, dtype=np.float32).astype(
        ml_dtypes.bfloat16)
    w1t, w2t, b1t, b2c = _prep_weights(
        np.asarray(W1, np.float32), np.asarray(b1, np.float32),
        np.asarray(W2, np.float32), np.asarray(b2, np.float32))

    if "nc" not in _NC_CACHE:
        _NC_CACHE["nc"] = build(BL)
    nc = _NC_CACHE["nc"]

    in_maps = []
    for i in range(N_CORES):
        r = slice(i * BL, (i + 1) * BL)
        in_maps.append({
            "sim_stance": sim_stance_emb[r],
            "nli_stance": nli_stance_emb[r],
            "sim_body": np.ascontiguousarray(
                sim_body_emb[r].reshape(BL, N * D)),
            "nli_body": np.ascontiguousarray(
                nli_body_emb[r].reshape(BL * N, D)),
            "w1t": w1t, "w2t": w2t, "b1t": b1t, "b2c": b2c,
        })

    res = run_bass_kernel_spmd(
        nc, in_maps, core_ids=list(range(N_CORES)),
        trace=_trace, tmpdir=_tmpdir,
    )
    out = np.concatenate([res.results[i]["out"] for i in range(N_CORES)], axis=0)
    if _trace:
        kernel.last_exec_time_ns = res.exec_time_ns
    return out


# revision 8
# speedup vs baseline: 1.2334x; 1.0426x over previous
"""Trainium2 Bass kernel for the AgreemFlat top-k masking model.

Reference computation (per batch row b):
    sims[b, n]  = <sim_stance_emb[b], sim_body_emb[b, n]>         n in [0, 64)
    top5        = top_k(sims[b], 5).indices                        (descending)
    xx[b]       = [nli_stance_emb[b] | nli_body_emb[b, top5].ravel()]   (4608)
    out[b]      = relu(xx @ W1.T + b1) @ W2.T + b2                 (4 classes)

Sharding: pure data-parallel over B=4096 -> 8 cores x 512 rows.
Weights replicated; W1^T / W2^T / b1 are pre-arranged host-side into
partition-major layouts so the device only does straight DMA loads.

Per-core dataflow (chunks of 128 batch rows on partitions):
  DMA sim_body stream -> DVE tensor_tensor_reduce (fp32 dot products)
  -> vector.max / max_index (top-8, take 5) -> indirect DMA gather
  -> PE transpose (+ ACT cast to bf16) -> PE matmul FC1 (bf16, fp32 psum)
  -> ACT fused bias+relu -> PE matmul FC2 -> bias add -> PE transpose -> DMA out.
"""

import numpy as np
import ml_dtypes

import concourse.bass as bass
import concourse.mybir as mybir
import concourse.tile as tile
from concourse import bacc
from concourse.masks import make_identity

P = 128
N = 64            # body sentences per row
D = 768           # embedding dim (sim and nli)
KK = 5            # top-k
H1 = 1024         # fc1 output dim
NC_OUT = 4        # classes
FAN1 = (KK + 1) * D          # 4608 = fc1 fan-in
KCH = FAN1 // P              # 36 contraction chunks
HCH = H1 // P                # 8 fc1-output chunks
N_CORES = 8
B_FULL = 4096
BL = B_FULL // N_CORES       # 512 rows per core
NSUB = 8                     # body n-sentences per streamed subtile


def build(bl=BL):
    chunks = bl // P
    nsubs = N // NSUB
    fp32 = mybir.dt.float32
    bf16 = mybir.dt.bfloat16
    u32 = mybir.dt.uint32

    nc = bacc.Bacc("TRN2", target_bir_lowering=False)

    sim_stance = nc.dram_tensor("sim_stance", [bl, D], fp32, kind="ExternalInput")
    nli_stance = nc.dram_tensor("nli_stance", [bl, D], bf16, kind="ExternalInput")
    sim_body = nc.dram_tensor("sim_body", [bl, N * D], fp32, kind="ExternalInput")
    nli_body = nc.dram_tensor("nli_body", [bl * N, D], bf16, kind="ExternalInput")
    w1t = nc.dram_tensor("w1t", [P, KCH * H1], bf16, kind="ExternalInput")
    w2t = nc.dram_tensor("w2t", [P, HCH * NC_OUT], bf16, kind="ExternalInput")
    b1t = nc.dram_tensor("b1t", [P, HCH], fp32, kind="ExternalInput")
    b2c = nc.dram_tensor("b2c", [NC_OUT, 1], fp32, kind="ExternalInput")
    out = nc.dram_tensor("out", [bl, NC_OUT], fp32, kind="ExternalOutput")

    with tile.TileContext(nc) as tc:
        with (
            tc.tile_pool(name="wpool", bufs=1) as wpool,
            tc.tile_pool(name="bodyp", bufs=2) as bodyp,
            tc.tile_pool(name="stancep", bufs=2) as stancep,
            tc.tile_pool(name="simsp", bufs=2) as simsp,
            tc.tile_pool(name="topkp", bufs=2) as topkp,
            tc.tile_pool(name="gp", bufs=2) as gp,
            tc.tile_pool(name="xxtp", bufs=2) as xxtp,
            tc.tile_pool(name="yp", bufs=2) as yp,
            tc.tile_pool(name="op", bufs=2) as op,
            tc.tile_pool(name="pt", bufs=4, space="PSUM") as pt_pool,
            tc.tile_pool(name="py", bufs=2, space="PSUM") as py_pool,
            tc.tile_pool(name="po", bufs=1, space="PSUM") as po_pool,
            tc.tile_pool(name="pout", bufs=1, space="PSUM") as pout_pool,
        ):
            # ---- setup: weights, biases, identity ----
            w1t_sb = wpool.tile([P, KCH * H1], bf16)
            nc.gpsimd.dma_start(w1t_sb[:], w1t[:, :])
            w2t_sb = wpool.tile([P, HCH * NC_OUT], bf16)
            nc.gpsimd.dma_start(w2t_sb[:], w2t[:, :])
            b1t_sb = wpool.tile([P, HCH], fp32)
            nc.gpsimd.dma_start(b1t_sb[:], b1t[:, :])
            b2_sb = wpool.tile([NC_OUT, 1], fp32)
            nc.gpsimd.dma_start(b2_sb[:], b2c[:, :])
            ident = wpool.tile([P, P], fp32)
            make_identity(nc, ident[:])
            identb = wpool.tile([P, P], bf16)
            make_identity(nc, identb[:])

            for ch in range(chunks):
                r0 = ch * P
                # ---- stance tiles for this chunk ----
                sstance = stancep.tile([P, D], fp32, tag="sstance")
                nc.sync.dma_start(sstance[:], sim_stance[r0:r0 + P, :])
                nstance = stancep.tile([P, D], bf16, tag="nstance")
                nc.sync.dma_start(nstance[:], nli_stance[r0:r0 + P, :])

                # ---- sims: stream body, fused multiply+reduce on DVE ----
                # Final chunk tapers to smaller subtiles so the DVE tail after
                # the last DMA byte is short.
                if ch == chunks - 1:
                    sub_sizes = [NSUB] * (nsubs - 2) + [NSUB // 2] * 4
                else:
                    sub_sizes = [NSUB] * nsubs
                sims = simsp.tile([P, N], fp32)
                n0 = 0
                for sz in sub_sizes:
                    body = bodyp.tile([P, NSUB * D], fp32, tag="body")
                    nc.sync.dma_start(
                        body[:, :sz * D],
                        sim_body[r0:r0 + P, n0 * D:(n0 + sz) * D],
                    )
                    for j in range(sz):
                        nrow = n0 + j
                        seg = body[:, j * D:(j + 1) * D]
                        nc.vector.scalar_tensor_tensor(
                            out=seg,
                            in0=seg,
                            scalar=1.0,
                            in1=sstance[:],
                            op0=mybir.AluOpType.mult,
                            op1=mybir.AluOpType.mult,
                            accum_out=sims[:, nrow:nrow + 1],
                        )
                    n0 += sz

                # ---- top-5 of 64 (descending, matches jax.lax.top_k) ----
                max8 = topkp.tile([P, 8], fp32, tag="max8")
                nc.vector.max(out=max8[:], in_=sims[:])
                idx8 = topkp.tile([P, 8], u32, tag="idx8")
                nc.vector.max_index(out=idx8[:], in_max=max8[:], in_values=sims[:])

                # global row index into nli_body rows: (r0 + p) * 64 + idx
                rowbase = topkp.tile([P, 8], u32, tag="rowbase")
                nc.gpsimd.iota(
                    rowbase[:], [[0, 8]], base=r0 * N, channel_multiplier=N,
                )
                rows = topkp.tile([P, 8], u32, tag="rows")
                nc.vector.tensor_tensor(
                    out=rows[:], in0=idx8[:], in1=rowbase[:],
                    op=mybir.AluOpType.add,
                )

                # ---- gather top-5 nli body embeddings: [128, 5*768] ----
                xg = gp.tile([P, KK * D], bf16)
                for k in range(KK):
                    nc.gpsimd.indirect_dma_start(
                        out=xg[:, k * D:(k + 1) * D],
                        out_offset=None,
                        in_=nli_body[:, :],
                        in_offset=bass.IndirectOffsetOnAxis(
                            ap=rows[:, k:k + 1], axis=0
                        ),
                    )

                # ---- xx^T in bf16 via PE transpose + ACT cast ----
                # xx = [nli_stance | gathered] : [128, 4608]
                xxt = xxtp.tile([P, FAN1], bf16)
                for t in range(KCH):
                    if t < D // P:
                        src = nstance[:, t * P:(t + 1) * P]
                    else:
                        g = t - D // P
                        src = xg[:, g * P:(g + 1) * P]
                    ptile = pt_pool.tile([P, P], bf16, tag="pt")
                    nc.tensor.transpose(
                        out=ptile[:], in_=src, identity=identb[:])
                    nc.scalar.activation(
                        out=xxt[:, t * P:(t + 1) * P], in_=ptile[:],
                        func=mybir.ActivationFunctionType.Copy,
                    )

                # ---- FC1 (bf16): y^T[h, b] = W1 @ xx^T, fused bias+relu ----
                yt = yp.tile([P, H1], bf16)
                for m in range(HCH):
                    py = py_pool.tile([P, P], fp32, tag="py")
                    for c in range(KCH):
                        nc.tensor.matmul(
                            out=py[:],
                            lhsT=w1t_sb[:, c * H1 + m * P:c * H1 + (m + 1) * P],
                            rhs=xxt[:, c * P:(c + 1) * P],
                            start=(c == 0),
                            stop=(c == KCH - 1),
                        )
                    nc.scalar.activation(
                        out=yt[:, m * P:(m + 1) * P], in_=py[:],
                        func=mybir.ActivationFunctionType.Relu,
                        bias=b1t_sb[:, m:m + 1],
                    )

                # ---- FC2: o^T[4, b] = W2 @ y^T ----
                po = po_pool.tile([NC_OUT, P], fp32, tag="po")
                for m in range(HCH):
                    nc.tensor.matmul(
                        out=po[:],
                        lhsT=w2t_sb[:, m * NC_OUT:(m + 1) * NC_OUT],
                        rhs=yt[:, m * P:(m + 1) * P],
                        start=(m == 0),
                        stop=(m == HCH - 1),
                    )
                osb = op.tile([NC_OUT, P], fp32, tag="osb")
                nc.scalar.activation(
                    out=osb[:], in_=po[:],
                    func=mybir.ActivationFunctionType.Identity,
                    bias=b2_sb[:, 0:1],
                )

                # ---- transpose [4, 128] -> [128, 4] and store ----
                pout = pout_pool.tile([P, NC_OUT], fp32, tag="pout")
                nc.tensor.transpose(
                    out=pout[:], in_=osb[:], identity=ident[:NC_OUT, :NC_OUT]
                )
                out_sb = op.tile([P, NC_OUT], fp32, tag="outsb")
                nc.scalar.activation(
                    out=out_sb[:], in_=pout[:],
                    func=mybir.ActivationFunctionType.Copy,
                )
                nc.scalar.dma_start(out[r0:r0 + P, :], out_sb[:])

    nc.compile()
    return nc


def _prep_weights(W1, b1, W2, b2):
    bf16 = ml_dtypes.bfloat16
    w1t = (
        W1.T.reshape(KCH, P, H1).transpose(1, 0, 2).reshape(P, KCH * H1)
        .astype(bf16)
    )
    w2t = (
        W2.T.reshape(HCH, P, NC_OUT).transpose(1, 0, 2).reshape(P, HCH * NC_OUT)
        .astype(bf16)
    )
    b1t = np.ascontiguousarray(b1.reshape(HCH, P).T)
    b2c = np.ascontiguousarray(b2.reshape(NC_OUT, 1))
    return w1t, w2t, b1t, b2c


_NC_CACHE = {}


def kernel(sim_stance_emb, nli_stance_emb, sim_body_emb, nli_body_emb,
           W1, b1, W2, b2, _trace=False, _tmpdir=None):
    from concourse.bass_utils import run_bass_kernel_spmd

    sim_stance_emb = np.asarray(sim_stance_emb, dtype=np.float32)
    nli_stance_emb = np.asarray(nli_stance_emb, dtype=np.float32).astype(
        ml_dtypes.bfloat16)
    sim_body_emb = np.asarray(sim_body_emb, dtype=np.float32)
    nli_body_emb = np.asarray(nli_body_emb, dtype=np.float32).astype(
        ml_dtypes.bfloat16)
    w1t, w2t, b1t, b2c = _prep_weights(
        np.asarray(W1, np.float32), np.asarray(b1, np.float32),
        np.asarray(W2, np.float32), np.asarray(b2, np.float32))

    if "nc" not in _NC_CACHE:
        _NC_CACHE["nc"] = build(BL)
    nc = _NC_CACHE["nc"]

    in_maps = []
    for i in range(N_CORES):
        r = slice(i * BL, (i + 1) * BL)
        in_maps.append({
            "sim_stance": sim_stance_emb[r],
            "nli_stance": nli_stance_emb[r],
            "sim_body": np.ascontiguousarray(
                sim_body_emb[r].reshape(BL, N * D)),
            "nli_body": np.ascontiguousarray(
                nli_body_emb[r].reshape(BL * N, D)),
            "w1t": w1t, "w2t": w2t, "b1t": b1t, "b2c": b2c,
        })

    res = run_bass_kernel_spmd(
        nc, in_maps, core_ids=list(range(N_CORES)),
        trace=_trace, tmpdir=_tmpdir,
    )
    out = np.concatenate([res.results[i]["out"] for i in range(N_CORES)], axis=0)
    if _trace:
        kernel.last_exec_time_ns = res.exec_time_ns
    return out
